# revision 34
# baseline (speedup 1.0000x reference)
"""Causal multi-head self-attention on 8 Trainium2 NeuronCores.

Problem: X[4, 2048, 1024] fp32, W_Q/W_K/W_V/W_O [1024, 1024] fp32,
16 heads x 64 dims, causal softmax attention + output projection.

Sharding: core c handles batch b = c//2 and head-group g = c%2
(heads g*8..g*8+8, i.e. 512 of the 1024 channels).  Each core computes
its 8 heads' Q/K/V projections, causal attention, and a partial output
projection against W_O[:, g*512:(g+1)*512]; the host sums the two
partial outputs per batch (the "all-reduce after W_O" step).

Device kernel layout notes:
 - Score matmuls are 64-contraction and run as ROW-TILED PAIRS
   (tile 64x128 at row positions 0 and 64): head h2's K^T block
   [64, 128] against its Q rows [64, 512] for both heads of a channel
   chunk execute concurrently in the two halves of the PE array.
 - Q/K are produced transposed ([channels, tokens]); K needs no zero
   padding: kt[h2*64:(h2+1)*64, t] holds head h2's dims.
 - scores land in ONE psum tile sps[128 keys, 2 heads, 1024
   (2 key-chunks x 512 q)]; a single exp ACT covers both heads
   (halves the per-instruction ACT overhead).
 - Softmax skips the max-subtraction (scores are bounded ~|1.9| after
   the 1/8 scale, applied via the activation's free affine).
 - Causal masking multiplies the diagonal score blocks by a 0/1 mask
   after exp; exp/mask skip the fully-masked leading region of the
   second diagonal group.
 - V is stored [tokens, 512 ch + 64 ones]; using [V_head | ones] as the
   stationary operand of the P*V matmul makes PSUM rows 0..63 the
   unnormalized output and row 64 the softmax row-sums; normalization
   is 1/s = exp(-ln s) on ScalarE (both heads' sums batched into one
   [1, 2, 512] Ln+Exp pair), a DRAM-bounce broadcast, and VectorE
   multiplies.
 - The P*V matmuls for group g are issued AFTER group g+1's score
   matmuls (one-group software-pipeline skew) so the in-order tensor
   queue has score+filler work to run while group g+1's exp is on the
   scalar engine.
 - Projection/output-projection matmuls are spread between attention
   groups as fillers; output is stored bf16 (host accumulates in f32).
"""

import sys

if "/opt/trn_rl_repo" not in sys.path:
    sys.path.insert(0, "/opt/trn_rl_repo")

from contextlib import ExitStack

import ml_dtypes
import numpy as np

import concourse.bacc as bacc
import concourse.bass as bass
import concourse.hw_specs as _hw_specs
import concourse.tile as tile
from concourse import mybir
from concourse.bass_utils import run_bass_kernel_spmd

# Bias the activation-table chooser so Exp resolves to the set that also
# contains Ln ("natural_log_exp_and_others"): the kernel interleaves Exp
# (softmax) with Ln (reciprocal via exp(-ln s)), and per-function minimal
# sets would thrash the ~2.7us ACT table load on every switch.
_orig_get_activation_tables = _hw_specs.get_activation_tables


def _patched_activation_tables(arch):
    exp_fn = mybir.ActivationFunctionType.Exp
    out = {}
    for name, fns in _orig_get_activation_tables(arch).items():
        if name != "natural_log_exp_and_others" and exp_fn in fns:
            fns = [f for f in fns if f != exp_fn]
        out[name] = set(fns)
    return out


bacc.get_activation_tables = _patched_activation_tables

B = 4
S = 2048
D = 1024
H = 16
DH = 64

P = 128
DIN_C = D // P        # 8 contraction chunks for the projections
CC = 4                # channel chunks per core (512 / 128)
NHEAD = 8             # heads per core
QT = S // 512         # query tiles of 512
TT = S // 512         # token tiles of 512
VH = 65               # per-head V block: 64 dims + 1 ones column

F32R = mybir.dt.float32r
F32 = mybir.dt.float32
BF16 = mybir.dt.bfloat16
F8 = mybir.dt.float8e4
VHP = 80              # fp8 V head-block padded so the Ko step is 16B-aligned

LAST_RESULT = None
_NC_CACHE = None


def build_nc():
    nc = bacc.Bacc()

    xt_d = nc.dram_tensor("xt", [D, S], BF16, kind="ExternalInput")
    wqt_d = nc.dram_tensor("wqt", [D, 512], BF16, kind="ExternalInput")
    wkt_d = nc.dram_tensor("wkt", [D, 512], BF16, kind="ExternalInput")
    wvt_d = nc.dram_tensor("wvt", [D, 512], BF16, kind="ExternalInput")
    wot_d = nc.dram_tensor("wot", [512, D], BF16, kind="ExternalInput")
    mask_d = nc.dram_tensor("mask", [P, 2, 1024], BF16, kind="ExternalInput")
    yt_d = nc.dram_tensor("yt", [D, S], BF16, kind="ExternalOutput")

    xt_v = xt_d[:, :].rearrange("(kc p) t -> p kc t", p=P)
    wq_v = wqt_d[:, :].rearrange("(kc p) c -> p kc c", p=P)
    wk_v = wkt_d[:, :].rearrange("(kc p) c -> p kc c", p=P)
    wv_v = wvt_d[:, :].rearrange("(kc p) c -> p kc c", p=P)
    wot_v = wot_d[:, :].rearrange("(cc p) o -> p cc o", p=P)
    yt_v = yt_d[:, :]

    EXP = mybir.ActivationFunctionType.Exp

    with tile.TileContext(nc) as tc, ExitStack() as ctx:
        singles = ctx.enter_context(tc.tile_pool(name="singles", bufs=1))
        xt_pool = ctx.enter_context(tc.tile_pool(name="xtp", bufs=3))
        qk_pool = ctx.enter_context(tc.tile_pool(name="qkp", bufs=2))
        w_pool = ctx.enter_context(tc.tile_pool(name="wp", bufs=2))
        p_pool = ctx.enter_context(tc.tile_pool(name="pp", bufs=2))
        misc = ctx.enter_context(tc.tile_pool(name="misc", bufs=2))
        yt_pool = ctx.enter_context(tc.tile_pool(name="ytp", bufs=3))
        proj_ps = ctx.enter_context(tc.tile_pool(name="proj_ps", bufs=2, space="PSUM"))
        att_ps = ctx.enter_context(tc.tile_pool(name="att_ps", bufs=1, space="PSUM"))
        dram_pool = ctx.enter_context(tc.tile_pool(name="drp", bufs=2, space="DRAM"))

        v_sb = singles.tile([P, S // P, NHEAD, VH], BF16)
        ot_sb = singles.tile([P, CC, S], BF16)
        wot_sb = singles.tile([P, CC, D], BF16)
        mask_sb = singles.tile([P, 2, 1024], BF16)

        wv_sb = w_pool.tile([P, DIN_C, 512], BF16, tag="wv")

        qk_tiles = {}

        def make_qk(cc, qq=None, qk=None):
            qq = qq if qq is not None else nc.sync
            qk = qk if qk is not None else qq
            wq_sb = w_pool.tile([P, DIN_C, 128], BF16, tag="wq", name=f"wq_{cc}")
            wk_sb = w_pool.tile([P, DIN_C, 128], BF16, tag="wk", name=f"wk_{cc}")
            qq.dma_start(out=wq_sb, in_=wq_v[:, :, cc * 128:(cc + 1) * 128])
            qk.dma_start(out=wk_sb, in_=wk_v[:, :, cc * 128:(cc + 1) * 128])
            qt_sb = qk_pool.tile([P, S], BF16, tag="qt", name=f"qtsb_{cc}")
            # K^T stacked like Q: head h2's 64 dims live in partition rows
            # h2*64..h2*64+63 -> score matmuls are row-tiled 64x128 pairs.
            kt_sb = qk_pool.tile([P, S], BF16, tag="kt", name=f"ktsb_{cc}")
            qk_tiles[cc] = (wq_sb, wk_sb, qt_sb, kt_sb)

        def proj_chunks(cc, tt, xt_ready=None):
            """Emit the X-tile DMA now; return compute thunks (one PSUM
            group each) to interleave between attention groups."""
            wq_sb, wk_sb, qt_sb, kt_sb = qk_tiles[cc]
            if xt_ready is not None:
                xt_t = xt_ready
            else:
                xt_t = xt_pool.tile([P, DIN_C, 512], BF16, tag="xt",
                                    name=f"xt_{cc}_{tt}")
                nc.sync.dma_start(out=xt_t[:, 0:4, :],
                                  in_=xt_v[:, 0:4, tt * 512:(tt + 1) * 512])
                nc.sync.dma_start(out=xt_t[:, 4:8, :],
                                  in_=xt_v[:, 4:8, tt * 512:(tt + 1) * 512])
            thunks = []
            if cc == 0:
                for sub in range(4):
                    def vthunk(sub=sub, xt_t=xt_t, tt=tt):
                        vps = proj_ps.tile([P, 512], F32, tag="pp",
                                           name=f"vps_{tt}_{sub}")
                        for kc in range(DIN_C):
                            nc.tensor.matmul(
                                vps,
                                xt_t[:, kc, sub * 128:(sub + 1) * 128],
                                wv_sb[:, kc, :],
                                start=(kc == 0),
                                stop=(kc == DIN_C - 1),
                            )
                        nc.vector.tensor_copy(v_sb[:, tt * 4 + sub, :, 0:64], vps)
                    thunks.append(vthunk)

            def qthunk(xt_t=xt_t, tt=tt, cc=cc, wq_sb=wq_sb, qt_sb=qt_sb):
                qps = proj_ps.tile([P, 512], F32, tag="pp", name=f"qps_{cc}_{tt}")
                for kc in range(DIN_C):
                    nc.tensor.matmul(
                        qps, wq_sb[:, kc, :], xt_t[:, kc, :],
                        start=(kc == 0), stop=(kc == DIN_C - 1),
                    )
                nc.vector.tensor_copy(qt_sb[:, tt * 512:(tt + 1) * 512], qps)

            def kthunk(xt_t=xt_t, tt=tt, cc=cc, wk_sb=wk_sb, kt_sb=kt_sb):
                kps = proj_ps.tile([P, 512], F32, tag="pp", name=f"kps_{cc}_{tt}")
                for kc in range(DIN_C):
                    nc.tensor.matmul(
                        kps, wk_sb[:, kc, :], xt_t[:, kc, :],
                        start=(kc == 0), stop=(kc == DIN_C - 1),
                    )
                nc.vector.tensor_copy(kt_sb[:, tt * 512:(tt + 1) * 512], kps)

            # q/k first: their casts gate the next query tile's first score
            # matmuls, while v chunks are only read by later diagonal groups
            return [qthunk, kthunk] + thunks

        def oproj_chunk(tt_o, oc):
            def th():
                ops_o = proj_ps.tile([P, 512], F32, tag="pp",
                                     name=f"ops_o_{tt_o}_{oc}")
                for c2 in range(CC):
                    nc.tensor.matmul(
                        ops_o,
                        wot_sb[:, c2, oc * 128:(oc + 1) * 128],
                        ot_sb[:, c2, tt_o * 512:(tt_o + 1) * 512],
                        start=(c2 == 0),
                        stop=(c2 == CC - 1),
                    )
                y_t = yt_pool.tile([P, 512], BF16, tag="yt",
                                   name=f"yt_{tt_o}_{oc}")
                # the last tile's evacuations run when ScalarE is idle:
                # alternate engines so the copies don't serialize on DVE
                if tt_o == TT - 1 and oc % 2 == 1:
                    nc.scalar.copy(y_t, ops_o)
                else:
                    nc.vector.tensor_copy(y_t, ops_o)
                # alternate output queues (halves the final DMA drain); the
                # last tile's writes are split so the wire starts earlier
                yq = nc.sync if oc % 2 == 0 else nc.gpsimd
                if tt_o == TT - 1:
                    for h in range(2):
                        yq.dma_start(
                            out=yt_v[oc * 128:(oc + 1) * 128,
                                     tt_o * 512 + h * 256:
                                     tt_o * 512 + (h + 1) * 256],
                            in_=y_t[:, h * 256:(h + 1) * 256],
                        )
                else:
                    yq.dma_start(
                        out=yt_v[oc * 128:(oc + 1) * 128,
                                 tt_o * 512:(tt_o + 1) * 512],
                        in_=y_t,
                    )
            return th

        # ---- prologue: the first X tile streams in 4 chunks on the Sync
        # queue; Q/K/V weights go on the GpSimd queue in parallel so the
        # first projection matmuls can start after ~1.5us. ----
        xt_first = xt_pool.tile([P, DIN_C, 512], BF16, tag="xt", name="xt_0_0")
        make_qk(0, qq=nc.sync, qk=nc.gpsimd)
        for kc2 in range(4):
            nc.sync.dma_start(out=xt_first[:, 2 * kc2:2 * kc2 + 2, :],
                              in_=xt_v[:, 2 * kc2:2 * kc2 + 2, 0:512])
        nc.gpsimd.dma_start(out=wv_sb[:, 0:4, :], in_=wv_v[:, 0:4, :])
        nc.gpsimd.dma_start(out=wv_sb[:, 4:8, :], in_=wv_v[:, 4:8, :])
        pending = proj_chunks(0, 0, xt_ready=xt_first)
        nc.gpsimd.dma_start(out=mask_sb, in_=mask_d[:, :, :])
        # the V-projection copies fill the data columns; only col 64 of each
        # head block (the ones column for the P*V row-sum trick) is set here
        # (per token-chunk group so the V copies unblock progressively).
        for q4 in range(4):
            nc.gpsimd.memset(v_sb[:, q4 * 4:(q4 + 1) * 4, :, 64:65], 1.0)
        nc.sync.dma_start(out=wot_sb, in_=wot_v)
        # Q/K thunks first (their weights land first), then V.
        for th in pending:
            th()

        # Global filler pool: (deadline_iteration, cost_ns, thunk).  Thunks
        # carry over between iterations so the projection work (which is
        # supply-heavy at cc=0 thanks to the V thunks and at cc=3 thanks to
        # the output projection) pads the exp-latency windows of the
        # filler-poor iterations in between.
        fill_q = []
        INF = 99
        RATE = 1450  # ns of filler per attention group, ~global average

        def run_fill(i):
            _, _, fn = fill_q.pop(i)
            fn()

        emitted = [0]
        g_idx = [0]
        tail_res = []
        norm_pending = [None]

        for cc in range(CC):
            _, _, qt_sb, kt_sb = qk_tiles[cc]
            for qt in range(TT):
                it = cc * TT + qt
                if qt < TT - 1:
                    for th in proj_chunks(cc, qt + 1):
                        fill_q.append((it + 1, 1800, th))
                elif cc < CC - 1:
                    make_qk(cc + 1)
                    for th in proj_chunks(cc + 1, 0):
                        fill_q.append((it + 1, 1800, th))
                if cc == CC - 1 and qt >= 1:
                    # reserve a few chunks of the previous tile's output
                    # projection as tensor work for the final norm chain
                    n_res = 5 if qt == TT - 1 else 0
                    for oc in range(D // P - n_res):
                        fill_q.append((INF, 900, oproj_chunk(qt - 1, oc)))
                    tail_res += [oproj_chunk(qt - 1, oc)
                                 for oc in range(D // P - n_res, D // P)]

                # anything this iteration's attention reads must be emitted
                # before the score matmuls (emission order = engine order)
                while any(e[0] <= it for e in fill_q):
                    emitted[0] += fill_q[0][1]
                    run_fill(0)
                nd0 = sum(1 for e in fill_q if e[0] <= it + 1)
                nd_done = 0

                last_kc = 4 * qt + 3
                n_grps = 2 * qt + 2
                # both heads' P*V accumulators in ONE psum tile (one bank
                # per head) so the normalization Ln can read both row-sum
                # rows with a single PSUM-direct ACT.
                ops = att_ps.tile([P, 2, 512], F32, tag="ops",
                                  name=f"ops_{cc}_{qt}")
                pv_prev = None
                for grp in range(n_grps):
                    p_t = p_pool.tile([P, 2, 1024], BF16, tag="p",
                                      name=f"p_{cc}_{qt}_{grp}")
                    for j in range(2):
                        kc = grp * 2 + j
                        # per-j psum + exp: halves the exp latency that
                        # serializes (via the sps WAR) with the next
                        # group's score matmuls
                        sps = att_ps.tile([P, 2, 512], F32, tag=f"sps{j}",
                                          name=f"sps{j}_{cc}_{qt}_{grp}")
                        for h2 in range(2):
                            # 64-contraction row-tiled pair: h2=0 in array
                            # rows 0-63, h2=1 in rows 64-127, concurrent.
                            nc.tensor.matmul(
                                sps[:, h2, :],
                                kt_sb[h2 * 64:(h2 + 1) * 64,
                                      kc * 128:(kc + 1) * 128],
                                qt_sb[h2 * 64:(h2 + 1) * 64,
                                      qt * 512:(qt + 1) * 512],
                                start=True,
                                stop=True,
                            )
                        # fully-masked leading columns are never read by
                        # the trimmed P*V matmuls: skip them in exp/mask
                        qlo = max(0, kc * 128 - qt * 512)
                        nc.scalar.activation(
                            p_t[:, :, j * 512 + qlo:(j + 1) * 512],
                            sps[:, :, qlo:], EXP, scale=0.125)
                        if grp >= 2 * qt:  # diagonal: causal mask
                            mv = grp - 2 * qt
                            for h2 in range(2):
                                nc.vector.tensor_mul(
                                    p_t[:, h2, j * 512 + qlo:(j + 1) * 512],
                                    p_t[:, h2, j * 512 + qlo:(j + 1) * 512],
                                    mask_sb[:, mv, j * 512 + qlo:(j + 1) * 512],
                                )

                    def pv_thunk(grp=grp, p_t=p_t, cc=cc, qt=qt,
                                 last_kc=last_kc):
                        for j in range(2):
                            kc = grp * 2 + j
                            qlo = max(0, kc * 128 - qt * 512)
                            for h2 in range(2):
                                nc.tensor.matmul(
                                    ops[0:VH, h2, qlo:512],
                                    v_sb[:, kc, 2 * cc + h2, 0:VH],
                                    p_t[:, h2, j * 512 + qlo:(j + 1) * 512],
                                    start=(kc == 0),
                                    stop=(kc == last_kc),
                                    skip_group_check=True,
                                )

                    # previous iteration's deferred normalization: its
                    # Ln/Exp land on the ACT queue AFTER this group's exps
                    if norm_pending[0] is not None:
                        norm_pending[0]()
                        norm_pending[0] = None
                    # one-group skew: run the PREVIOUS group's P*V now, so
                    # the tensor queue isn't head-blocked on this group's
                    # exp; fillers (projection work) pad the rest.
                    if pv_prev is not None:
                        pv_prev()
                    pv_prev = pv_thunk
                    g_idx[0] += 1
                    # pace the pool: global rate, plus spread next
                    # iteration's deadline thunks across this one's groups
                    nd_want = nd0 * (grp + 1 + n_grps // 2) // n_grps
                    while fill_q and (
                            emitted[0] < g_idx[0] * RATE
                            or len(fill_q) > 8
                            or (nd_done < nd_want and fill_q[0][0] <= it + 1)):
                        if fill_q[0][0] <= it + 1:
                            nd_done += 1
                        emitted[0] += fill_q[0][1]
                        run_fill(0)
                pv_prev()

                # ---- normalization: U / s with s from the ones column.
                # Ln reads the PSUM row-sum rows directly; the chain is
                # EMITTED inside the next iteration's first group so its
                # ACT pair doesn't delay that iteration's first exps. ----
                def emit_norm(cc=cc, qt=qt, ops=ops, last=False):
                    rec_s = misc.tile([VH, 2, 512], F32, tag="recs",
                                      name=f"recs_{cc}_{qt}")
                    nc.scalar.activation(rec_s[64:65, :, :],
                                         ops[64:65, :, :],
                                         mybir.ActivationFunctionType.Ln)
                    u_sb = misc.tile([VH, 2, 512], F32, tag="u",
                                     name=f"u_{cc}_{qt}")
                    nc.vector.tensor_copy(u_sb, ops[0:VH, :, :])
                    rec_e = misc.tile([VH, 2, 512], F32, tag="rece",
                                      name=f"rece_{cc}_{qt}")
                    nc.scalar.activation(rec_e[64:65, :, :],
                                         rec_s[64:65, :, :], EXP, scale=-1.0)
                    # broadcast 1/s across the 64 head dims via a DRAM
                    # bounce (SBUF sources cannot have partition-stride 0)
                    rdram = dram_pool.tile([1, 2, 512], F32, tag="rd",
                                           name=f"rd_{cc}_{qt}")
                    nc.gpsimd.dma_start(out=rdram, in_=rec_e[64:65, :, :])
                    rec = misc.tile([64, 2, 512], F32, tag="rec",
                                    name=f"rec_{cc}_{qt}")
                    for h2 in range(2):
                        rsrc = rdram[0:1, h2, :]
                        nc.gpsimd.dma_start(
                            out=rec[:, h2, :],
                            in_=bass.AP(tensor=rsrc.tensor,
                                        offset=rsrc.offset,
                                        ap=[[0, 64], [1, 512]]),
                        )
                    if last:
                        for th in tail_res:
                            th()
                    for h2 in range(2):
                        nc.vector.tensor_mul(
                            ot_sb[h2 * 64:h2 * 64 + 64, cc,
                                  qt * 512:(qt + 1) * 512],
                            u_sb[0:64, h2, :],
                            rec[:, h2, :],
                        )
                if cc == CC - 1 and qt == TT - 1:
                    emit_norm(last=True)
                else:
                    norm_pending[0] = emit_norm

        # tail: drain the filler pool, then the last token-tile's output
        # projection (its norm chain latency is covered by the drain)
        while fill_q:
            run_fill(0)
        for oc in range(D // P):
            oproj_chunk(TT - 1, oc)()

    nc.finalize()
    return nc


def _make_mask():
    # variant v covers key blocks 2v,2v+1 (128 keys each) of the diagonal
    # 512-query window: mask[k, v, j*512+q] = (v*256 + j*128 + k <= q)
    keys = (np.arange(2)[None, :, None, None] * 256
            + np.arange(2)[None, None, :, None] * 128
            + np.arange(128)[:, None, None, None])
    qs = np.arange(512)[None, None, None, :]
    return (keys <= qs).astype(np.float32).reshape(128, 2, 1024)


def kernel(X, W_Q, W_K, W_V, W_O):
    global LAST_RESULT, _NC_CACHE
    X = np.asarray(X, dtype=np.float32)
    W_Q = np.asarray(W_Q, dtype=np.float32)
    W_K = np.asarray(W_K, dtype=np.float32)
    W_V = np.asarray(W_V, dtype=np.float32)
    W_O = np.asarray(W_O, dtype=np.float32)

    mask = _make_mask().astype(ml_dtypes.bfloat16)
    in_maps = []
    for c in range(8):
        b, g = c // 2, c % 2
        sl = slice(g * 512, (g + 1) * 512)
        in_maps.append({
            "xt": np.ascontiguousarray(X[b].T).astype(ml_dtypes.bfloat16),
            "wqt": np.ascontiguousarray(W_Q[sl, :].T).astype(ml_dtypes.bfloat16),
            "wkt": np.ascontiguousarray(W_K[sl, :].T).astype(ml_dtypes.bfloat16),
            "wvt": np.ascontiguousarray(W_V[sl, :].T).astype(ml_dtypes.bfloat16),
            "wot": np.ascontiguousarray(W_O[:, sl].T).astype(ml_dtypes.bfloat16),
            "mask": mask,
        })

    if _NC_CACHE is None:
        _NC_CACHE = build_nc()
    res = run_bass_kernel_spmd(_NC_CACHE, in_maps, core_ids=list(range(8)))
    LAST_RESULT = res

    out = np.empty((B, S, D), dtype=np.float32)
    for b in range(B):
        yt = (res.results[2 * b]["yt"].astype(np.float32)
              + res.results[2 * b + 1]["yt"].astype(np.float32))
        out[b] = yt.T
    return out


# revision 35
# speedup vs baseline: 1.0222x; 1.0222x over previous
"""Causal multi-head self-attention on 8 Trainium2 NeuronCores.

Problem: X[4, 2048, 1024] fp32, W_Q/W_K/W_V/W_O [1024, 1024] fp32,
16 heads x 64 dims, causal softmax attention + output projection.

Sharding: core c handles batch b = c//2 and head-group g = c%2
(heads g*8..g*8+8, i.e. 512 of the 1024 channels).  Each core computes
its 8 heads' Q/K/V projections, causal attention, and a partial output
projection against W_O[:, g*512:(g+1)*512]; the host sums the two
partial outputs per batch (the "all-reduce after W_O" step).

Device kernel layout notes:
 - Score matmuls are 64-contraction and run as ROW-TILED PAIRS
   (tile 64x128 at row positions 0 and 64): head h2's K^T block
   [64, 128] against its Q rows [64, 512] for both heads of a channel
   chunk execute concurrently in the two halves of the PE array.
 - Q/K are produced transposed ([channels, tokens]); K needs no zero
   padding: kt[h2*64:(h2+1)*64, t] holds head h2's dims.
 - scores land in ONE psum tile sps[128 keys, 2 heads, 1024
   (2 key-chunks x 512 q)]; a single exp ACT covers both heads
   (halves the per-instruction ACT overhead).
 - Softmax skips the max-subtraction (scores are bounded ~|1.9| after
   the 1/8 scale, applied via the activation's free affine).
 - Causal masking multiplies the diagonal score blocks by a 0/1 mask
   after exp; exp/mask skip the fully-masked leading region of the
   second diagonal group.
 - V is stored [tokens, 512 ch + 64 ones]; using [V_head | ones] as the
   stationary operand of the P*V matmul makes PSUM rows 0..63 the
   unnormalized output and row 64 the softmax row-sums; normalization
   is 1/s = exp(-ln s) on ScalarE (both heads' sums batched into one
   [1, 2, 512] Ln+Exp pair), a DRAM-bounce broadcast, and VectorE
   multiplies.
 - The P*V matmuls for group g are issued AFTER group g+1's score
   matmuls (one-group software-pipeline skew) so the in-order tensor
   queue has score+filler work to run while group g+1's exp is on the
   scalar engine.
 - Projection/output-projection matmuls are spread between attention
   groups as fillers; output is stored bf16 (host accumulates in f32).
"""

import sys

if "/opt/trn_rl_repo" not in sys.path:
    sys.path.insert(0, "/opt/trn_rl_repo")

from contextlib import ExitStack

import ml_dtypes
import numpy as np

import concourse.bacc as bacc
import concourse.bass as bass
import concourse.hw_specs as _hw_specs
import concourse.tile as tile
from concourse import mybir
from concourse.bass_utils import run_bass_kernel_spmd

# Bias the activation-table chooser so Exp resolves to the set that also
# contains Ln ("natural_log_exp_and_others"): the kernel interleaves Exp
# (softmax) with Ln (reciprocal via exp(-ln s)), and per-function minimal
# sets would thrash the ~2.7us ACT table load on every switch.
_orig_get_activation_tables = _hw_specs.get_activation_tables


def _patched_activation_tables(arch):
    exp_fn = mybir.ActivationFunctionType.Exp
    out = {}
    for name, fns in _orig_get_activation_tables(arch).items():
        if name != "natural_log_exp_and_others" and exp_fn in fns:
            fns = [f for f in fns if f != exp_fn]
        out[name] = set(fns)
    return out


bacc.get_activation_tables = _patched_activation_tables

B = 4
S = 2048
D = 1024
H = 16
DH = 64

P = 128
DIN_C = D // P        # 8 contraction chunks for the projections
CC = 4                # channel chunks per core (512 / 128)
NHEAD = 8             # heads per core
QT = S // 512         # query tiles of 512
TT = S // 512         # token tiles of 512
VH = 65               # per-head V block: 64 dims + 1 ones column

F32R = mybir.dt.float32r
F32 = mybir.dt.float32
BF16 = mybir.dt.bfloat16
F8 = mybir.dt.float8e4
VHP = 80              # fp8 V head-block padded so the Ko step is 16B-aligned

LAST_RESULT = None
_NC_CACHE = None


def build_nc():
    nc = bacc.Bacc()

    xt_d = nc.dram_tensor("xt", [D, S], BF16, kind="ExternalInput")
    wqt_d = nc.dram_tensor("wqt", [D, 512], BF16, kind="ExternalInput")
    wkt_d = nc.dram_tensor("wkt", [D, 512], BF16, kind="ExternalInput")
    wvt_d = nc.dram_tensor("wvt", [D, 512], BF16, kind="ExternalInput")
    wot_d = nc.dram_tensor("wot", [512, D], BF16, kind="ExternalInput")
    mask_d = nc.dram_tensor("mask", [P, 2, 1024], BF16, kind="ExternalInput")
    yt_d = nc.dram_tensor("yt", [D, S], BF16, kind="ExternalOutput")

    xt_v = xt_d[:, :].rearrange("(kc p) t -> p kc t", p=P)
    wq_v = wqt_d[:, :].rearrange("(kc p) c -> p kc c", p=P)
    wk_v = wkt_d[:, :].rearrange("(kc p) c -> p kc c", p=P)
    wv_v = wvt_d[:, :].rearrange("(kc p) c -> p kc c", p=P)
    wot_v = wot_d[:, :].rearrange("(cc p) o -> p cc o", p=P)
    yt_v = yt_d[:, :]

    EXP = mybir.ActivationFunctionType.Exp

    with tile.TileContext(nc) as tc, ExitStack() as ctx:
        singles = ctx.enter_context(tc.tile_pool(name="singles", bufs=1))
        xt_pool = ctx.enter_context(tc.tile_pool(name="xtp", bufs=3))
        qk_pool = ctx.enter_context(tc.tile_pool(name="qkp", bufs=2))
        w_pool = ctx.enter_context(tc.tile_pool(name="wp", bufs=2))
        p_pool = ctx.enter_context(tc.tile_pool(name="pp", bufs=2))
        misc = ctx.enter_context(tc.tile_pool(name="misc", bufs=2))
        yt_pool = ctx.enter_context(tc.tile_pool(name="ytp", bufs=3))
        proj_ps = ctx.enter_context(tc.tile_pool(name="proj_ps", bufs=2, space="PSUM"))
        att_ps = ctx.enter_context(tc.tile_pool(name="att_ps", bufs=1, space="PSUM"))
        dram_pool = ctx.enter_context(tc.tile_pool(name="drp", bufs=2, space="DRAM"))

        v_sb = singles.tile([P, S // P, NHEAD, VH], BF16)
        ot_sb = singles.tile([P, CC, S], BF16)
        wot_sb = singles.tile([P, CC, D], BF16)
        mask_sb = singles.tile([P, 2, 1024], BF16)

        wv_sb = w_pool.tile([P, DIN_C, 512], BF16, tag="wv")

        qk_tiles = {}

        def make_qk(cc, qq=None, qk=None):
            qq = qq if qq is not None else nc.sync
            qk = qk if qk is not None else qq
            wq_sb = w_pool.tile([P, DIN_C, 128], BF16, tag="wq", name=f"wq_{cc}")
            wk_sb = w_pool.tile([P, DIN_C, 128], BF16, tag="wk", name=f"wk_{cc}")
            qq.dma_start(out=wq_sb, in_=wq_v[:, :, cc * 128:(cc + 1) * 128])
            qk.dma_start(out=wk_sb, in_=wk_v[:, :, cc * 128:(cc + 1) * 128])
            qt_sb = qk_pool.tile([P, S], BF16, tag="qt", name=f"qtsb_{cc}")
            # K^T stacked like Q: head h2's 64 dims live in partition rows
            # h2*64..h2*64+63 -> score matmuls are row-tiled 64x128 pairs.
            kt_sb = qk_pool.tile([P, S], BF16, tag="kt", name=f"ktsb_{cc}")
            qk_tiles[cc] = (wq_sb, wk_sb, qt_sb, kt_sb)

        def proj_chunks(cc, tt, xt_ready=None):
            """Emit the X-tile DMA now; return compute thunks (one PSUM
            group each) to interleave between attention groups."""
            wq_sb, wk_sb, qt_sb, kt_sb = qk_tiles[cc]
            if xt_ready is not None:
                xt_t = xt_ready
            else:
                xt_t = xt_pool.tile([P, DIN_C, 512], BF16, tag="xt",
                                    name=f"xt_{cc}_{tt}")
                nc.sync.dma_start(out=xt_t[:, 0:4, :],
                                  in_=xt_v[:, 0:4, tt * 512:(tt + 1) * 512])
                nc.sync.dma_start(out=xt_t[:, 4:8, :],
                                  in_=xt_v[:, 4:8, tt * 512:(tt + 1) * 512])
            thunks = []
            if cc == 0:
                for sub in range(4):
                    def vthunk(sub=sub, xt_t=xt_t, tt=tt):
                        vps = proj_ps.tile([P, 512], F32, tag="pp",
                                           name=f"vps_{tt}_{sub}")
                        for kc in range(DIN_C):
                            nc.tensor.matmul(
                                vps,
                                xt_t[:, kc, sub * 128:(sub + 1) * 128],
                                wv_sb[:, kc, :],
                                start=(kc == 0),
                                stop=(kc == DIN_C - 1),
                            )
                        nc.vector.tensor_copy(v_sb[:, tt * 4 + sub, :, 0:64], vps)
                    thunks.append(vthunk)

            def qthunk(xt_t=xt_t, tt=tt, cc=cc, wq_sb=wq_sb, qt_sb=qt_sb):
                qps = proj_ps.tile([P, 512], F32, tag="pp", name=f"qps_{cc}_{tt}")
                for kc in range(DIN_C):
                    nc.tensor.matmul(
                        qps, wq_sb[:, kc, :], xt_t[:, kc, :],
                        start=(kc == 0), stop=(kc == DIN_C - 1),
                    )
                nc.vector.tensor_copy(qt_sb[:, tt * 512:(tt + 1) * 512], qps)

            def kthunk(xt_t=xt_t, tt=tt, cc=cc, wk_sb=wk_sb, kt_sb=kt_sb):
                kps = proj_ps.tile([P, 512], F32, tag="pp", name=f"kps_{cc}_{tt}")
                for kc in range(DIN_C):
                    nc.tensor.matmul(
                        kps, wk_sb[:, kc, :], xt_t[:, kc, :],
                        start=(kc == 0), stop=(kc == DIN_C - 1),
                    )
                nc.vector.tensor_copy(kt_sb[:, tt * 512:(tt + 1) * 512], kps)

            # q/k first: their casts gate the next query tile's first score
            # matmuls, while v chunks are only read by later diagonal groups
            return [qthunk, kthunk] + thunks

        def oproj_chunk(tt_o, oc):
            def th():
                ops_o = proj_ps.tile([P, 512], F32, tag="pp",
                                     name=f"ops_o_{tt_o}_{oc}")
                for c2 in range(CC):
                    nc.tensor.matmul(
                        ops_o,
                        wot_sb[:, c2, oc * 128:(oc + 1) * 128],
                        ot_sb[:, c2, tt_o * 512:(tt_o + 1) * 512],
                        start=(c2 == 0),
                        stop=(c2 == CC - 1),
                    )
                y_t = yt_pool.tile([P, 512], BF16, tag="yt",
                                   name=f"yt_{tt_o}_{oc}")
                nc.vector.tensor_copy(y_t, ops_o)
                # alternate output queues so the final DMA drain halves
                yq = nc.sync if oc % 2 == 0 else nc.gpsimd
                yq.dma_start(
                    out=yt_v[oc * 128:(oc + 1) * 128,
                             tt_o * 512:(tt_o + 1) * 512],
                    in_=y_t,
                )
            return th

        # ---- prologue: the first X tile streams in 4 chunks on the Sync
        # queue; Q/K/V weights go on the GpSimd queue in parallel so the
        # first projection matmuls can start after ~1.5us. ----
        xt_first = xt_pool.tile([P, DIN_C, 512], BF16, tag="xt", name="xt_0_0")
        make_qk(0, qq=nc.sync, qk=nc.gpsimd)
        for kc2 in range(4):
            nc.sync.dma_start(out=xt_first[:, 2 * kc2:2 * kc2 + 2, :],
                              in_=xt_v[:, 2 * kc2:2 * kc2 + 2, 0:512])
        nc.gpsimd.dma_start(out=wv_sb[:, 0:4, :], in_=wv_v[:, 0:4, :])
        nc.gpsimd.dma_start(out=wv_sb[:, 4:8, :], in_=wv_v[:, 4:8, :])
        pending = proj_chunks(0, 0, xt_ready=xt_first)
        nc.gpsimd.dma_start(out=mask_sb, in_=mask_d[:, :, :])
        # the V-projection copies fill the data columns; only col 64 of each
        # head block (the ones column for the P*V row-sum trick) is set here
        # (per token-chunk group so the V copies unblock progressively).
        for q4 in range(4):
            nc.gpsimd.memset(v_sb[:, q4 * 4:(q4 + 1) * 4, :, 64:65], 1.0)
        nc.sync.dma_start(out=wot_sb, in_=wot_v)
        # Q/K thunks first (their weights land first), then V.
        for th in pending:
            th()

        # Global filler pool: (deadline_iteration, cost_ns, thunk).  Thunks
        # carry over between iterations so the projection work (which is
        # supply-heavy at cc=0 thanks to the V thunks and at cc=3 thanks to
        # the output projection) pads the exp-latency windows of the
        # filler-poor iterations in between.
        fill_q = []
        INF = 99
        RATE = 1450  # ns of filler per attention group, ~global average

        def run_fill(i):
            _, _, fn = fill_q.pop(i)
            fn()

        emitted = [0]
        g_idx = [0]
        tail_res = []

        for cc in range(CC):
            _, _, qt_sb, kt_sb = qk_tiles[cc]
            for qt in range(TT):
                it = cc * TT + qt
                if qt < TT - 1:
                    for th in proj_chunks(cc, qt + 1):
                        fill_q.append((it + 1, 1800, th))
                elif cc < CC - 1:
                    make_qk(cc + 1)
                    for th in proj_chunks(cc + 1, 0):
                        fill_q.append((it + 1, 1800, th))
                if cc == CC - 1 and qt >= 1:
                    # reserve a few chunks of the previous tile's output
                    # projection as tensor work for the final norm chain
                    n_res = 3 if qt == TT - 1 else 0
                    for oc in range(D // P - n_res):
                        fill_q.append((INF, 900, oproj_chunk(qt - 1, oc)))
                    tail_res += [oproj_chunk(qt - 1, oc)
                                 for oc in range(D // P - n_res, D // P)]

                # anything this iteration's attention reads must be emitted
                # before the score matmuls (emission order = engine order)
                while any(e[0] <= it for e in fill_q):
                    emitted[0] += fill_q[0][1]
                    run_fill(0)
                nd0 = sum(1 for e in fill_q if e[0] <= it + 1)
                nd_done = 0

                last_kc = 4 * qt + 3
                n_grps = 2 * qt + 2
                # both heads' P*V accumulators in ONE psum tile (one bank
                # per head) so the normalization Ln can read both row-sum
                # rows with a single PSUM-direct ACT.
                ops = att_ps.tile([P, 2, 512], F32, tag="ops",
                                  name=f"ops_{cc}_{qt}")
                pv_prev = None
                for grp in range(n_grps):
                    p_t = p_pool.tile([P, 2, 1024], BF16, tag="p",
                                      name=f"p_{cc}_{qt}_{grp}")
                    for j in range(2):
                        kc = grp * 2 + j
                        # per-j psum + exp: halves the exp latency that
                        # serializes (via the sps WAR) with the next
                        # group's score matmuls
                        sps = att_ps.tile([P, 2, 512], F32, tag=f"sps{j}",
                                          name=f"sps{j}_{cc}_{qt}_{grp}")
                        for h2 in range(2):
                            # 64-contraction row-tiled pair: h2=0 in array
                            # rows 0-63, h2=1 in rows 64-127, concurrent.
                            nc.tensor.matmul(
                                sps[:, h2, :],
                                kt_sb[h2 * 64:(h2 + 1) * 64,
                                      kc * 128:(kc + 1) * 128],
                                qt_sb[h2 * 64:(h2 + 1) * 64,
                                      qt * 512:(qt + 1) * 512],
                                start=True,
                                stop=True,
                            )
                        # fully-masked leading columns are never read by
                        # the trimmed P*V matmuls: skip them in exp/mask
                        qlo = max(0, kc * 128 - qt * 512)
                        nc.scalar.activation(
                            p_t[:, :, j * 512 + qlo:(j + 1) * 512],
                            sps[:, :, qlo:], EXP, scale=0.125)
                        if grp >= 2 * qt:  # diagonal: causal mask
                            mv = grp - 2 * qt
                            for h2 in range(2):
                                nc.vector.tensor_mul(
                                    p_t[:, h2, j * 512 + qlo:(j + 1) * 512],
                                    p_t[:, h2, j * 512 + qlo:(j + 1) * 512],
                                    mask_sb[:, mv, j * 512 + qlo:(j + 1) * 512],
                                )

                    def pv_thunk(grp=grp, p_t=p_t, cc=cc, qt=qt,
                                 last_kc=last_kc):
                        for j in range(2):
                            kc = grp * 2 + j
                            qlo = max(0, kc * 128 - qt * 512)
                            for h2 in range(2):
                                nc.tensor.matmul(
                                    ops[0:VH, h2, qlo:512],
                                    v_sb[:, kc, 2 * cc + h2, 0:VH],
                                    p_t[:, h2, j * 512 + qlo:(j + 1) * 512],
                                    start=(kc == 0),
                                    stop=(kc == last_kc),
                                    skip_group_check=True,
                                )

                    # one-group skew: run the PREVIOUS group's P*V now, so
                    # the tensor queue isn't head-blocked on this group's
                    # exp; fillers (projection work) pad the rest.
                    if pv_prev is not None:
                        pv_prev()
                    pv_prev = pv_thunk
                    g_idx[0] += 1
                    # pace the pool: global rate, plus spread next
                    # iteration's deadline thunks across this one's groups
                    nd_want = nd0 * (grp + 1 + n_grps // 2) // n_grps
                    while fill_q and (
                            emitted[0] < g_idx[0] * RATE
                            or (nd_done < nd_want and fill_q[0][0] <= it + 1)):
                        if fill_q[0][0] <= it + 1:
                            nd_done += 1
                        emitted[0] += fill_q[0][1]
                        run_fill(0)
                pv_prev()

                # ---- normalization: U / s with s from the ones column.
                # Ln reads the PSUM row-sum rows directly (single ACT, both
                # heads) so the reciprocal chain starts without waiting for
                # the U copy, which runs concurrently on VectorE. ----
                rec_s = misc.tile([VH, 2, 512], F32, tag="recs",
                                  name=f"recs_{cc}_{qt}")
                nc.scalar.activation(rec_s[64:65, :, :], ops[64:65, :, :],
                                     mybir.ActivationFunctionType.Ln)
                u_sb = misc.tile([VH, 2, 512], F32, tag="u",
                                 name=f"u_{cc}_{qt}")
                nc.vector.tensor_copy(u_sb, ops[0:VH, :, :])
                rec_e = misc.tile([VH, 2, 512], F32, tag="rece",
                                  name=f"rece_{cc}_{qt}")
                nc.scalar.activation(rec_e[64:65, :, :], rec_s[64:65, :, :],
                                     EXP, scale=-1.0)
                # broadcast 1/s across the 64 head dims via a DRAM bounce
                # (SBUF sources cannot have partition-stride 0)
                rdram = dram_pool.tile([1, 2, 512], F32, tag="rd",
                                       name=f"rd_{cc}_{qt}")
                nc.gpsimd.dma_start(out=rdram, in_=rec_e[64:65, :, :])
                rec = misc.tile([64, 2, 512], F32, tag="rec",
                                name=f"rec_{cc}_{qt}")
                for h2 in range(2):
                    rsrc = rdram[0:1, h2, :]
                    nc.gpsimd.dma_start(
                        out=rec[:, h2, :],
                        in_=bass.AP(tensor=rsrc.tensor, offset=rsrc.offset,
                                    ap=[[0, 64], [1, 512]]),
                    )
                if cc == CC - 1 and qt == TT - 1:
                    for th in tail_res:
                        th()
                for h2 in range(2):
                    nc.vector.tensor_mul(
                        ot_sb[h2 * 64:h2 * 64 + 64, cc,
                              qt * 512:(qt + 1) * 512],
                        u_sb[0:64, h2, :],
                        rec[:, h2, :],
                    )

        # tail: drain the filler pool, then the last token-tile's output
        # projection (its norm chain latency is covered by the drain)
        while fill_q:
            run_fill(0)
        for oc in range(D // P):
            oproj_chunk(TT - 1, oc)()

    nc.finalize()
    return nc


def _make_mask():
    # variant v covers key blocks 2v,2v+1 (128 keys each) of the diagonal
    # 512-query window: mask[k, v, j*512+q] = (v*256 + j*128 + k <= q)
    keys = (np.arange(2)[None, :, None, None] * 256
            + np.arange(2)[None, None, :, None] * 128
            + np.arange(128)[:, None, None, None])
    qs = np.arange(512)[None, None, None, :]
    return (keys <= qs).astype(np.float32).reshape(128, 2, 1024)


def kernel(X, W_Q, W_K, W_V, W_O):
    global LAST_RESULT, _NC_CACHE
    X = np.asarray(X, dtype=np.float32)
    W_Q = np.asarray(W_Q, dtype=np.float32)
    W_K = np.asarray(W_K, dtype=np.float32)
    W_V = np.asarray(W_V, dtype=np.float32)
    W_O = np.asarray(W_O, dtype=np.float32)

    mask = _make_mask().astype(ml_dtypes.bfloat16)
    in_maps = []
    for c in range(8):
        b, g = c // 2, c % 2
        sl = slice(g * 512, (g + 1) * 512)
        in_maps.append({
            "xt": np.ascontiguousarray(X[b].T).astype(ml_dtypes.bfloat16),
            "wqt": np.ascontiguousarray(W_Q[sl, :].T).astype(ml_dtypes.bfloat16),
            "wkt": np.ascontiguousarray(W_K[sl, :].T).astype(ml_dtypes.bfloat16),
            "wvt": np.ascontiguousarray(W_V[sl, :].T).astype(ml_dtypes.bfloat16),
            "wot": np.ascontiguousarray(W_O[:, sl].T).astype(ml_dtypes.bfloat16),
            "mask": mask,
        })

    if _NC_CACHE is None:
        _NC_CACHE = build_nc()
    res = run_bass_kernel_spmd(_NC_CACHE, in_maps, core_ids=list(range(8)))
    LAST_RESULT = res

    out = np.empty((B, S, D), dtype=np.float32)
    for b in range(B):
        yt = (res.results[2 * b]["yt"].astype(np.float32)
              + res.results[2 * b + 1]["yt"].astype(np.float32))
        out[b] = yt.T
    return out
